# revision 1
# baseline (speedup 1.0000x reference)
"""Trainium2 Bass kernel for CIDER ISAB title encoder.

x [2048, 32, 512] -> ISAB applied twice (shared params) -> mean over seq -> [2048, 512].
Data-parallel over 8 NeuronCores (256 batch elems each). float32r matmuls
(fp32 storage, tf32-like matmul rounding) with fp32 elementwise/LN/softmax.

Layout: activations canonical feature-major ([d partitions, tokens free]);
attention V tensors token-major; residual+LN islands token-major with PE
transposes at the boundaries. mab0 scores use the shared-Q trick (inducing
points are batch-independent); A^T for the AV matmuls is built with DVE
32x32 stream transposes into persistent zero-padded block-diagonal stores.
"""

import numpy as np

import concourse.bass as bass
import concourse.bacc as bacc
import concourse.mybir as mybir
from concourse.tile import TileContext
from concourse.masks import make_identity

F32 = mybir.dt.float32
F32R = mybir.dt.float32r
BF16 = mybir.dt.bfloat16
AF = mybir.ActivationFunctionType
ALU = mybir.AluOpType
AX = mybir.AxisListType

D = 512
DT = 4          # d tiles of 128
H = 8           # heads
HP = 4          # head pairs
M = 16          # inducing points
S = 32          # seq len
NCORES = 8
NB = 2048 // NCORES     # 256 batches per core
G = 16                  # batches per group
EPS = 1e-5
SCALE = 1.0 / np.sqrt(np.float32(D))


def _ap(base, extra_dims, extra_off=0):
    """Manual AP: keep base partition dim, replace free dims."""
    return bass.AP(base.tensor, base.offset + extra_off, [base.ap[0]] + extra_dims)


STAGE = 99


def build(nb=NB):
    ngroups = nb // G
    nc = bacc.Bacc(None, target_bir_lowering=False)

    x_d = nc.dram_tensor("x", [nb, S, D], F32, kind="ExternalInput")
    I_d = nc.dram_tensor("I", [1, M, D], F32, kind="ExternalInput")
    p_d = {}
    for mb in (0, 1):
        p_d[f"{mb}Wqkv"] = nc.dram_tensor(f"mab{mb}_Wqkv", [3, D, D], F32, kind="ExternalInput")
        p_d[f"{mb}bqkv"] = nc.dram_tensor(f"mab{mb}_bqkv", [3, D], F32, kind="ExternalInput")
        p_d[f"{mb}Wo"] = nc.dram_tensor(f"mab{mb}_Wo", [D, D], F32, kind="ExternalInput")
        p_d[f"{mb}bo"] = nc.dram_tensor(f"mab{mb}_bo", [D], F32, kind="ExternalInput")
        for nm in ("g0", "b0", "g1", "b1"):
            p_d[f"{mb}{nm}"] = nc.dram_tensor(f"mab{mb}_{nm}", [D], F32, kind="ExternalInput")
    out_d = nc.dram_tensor("out", [nb, D], F32, kind="ExternalOutput")

    with TileContext(nc) as tc:
        with tc.tile_pool(name="singles", bufs=1) as sg, \
             tc.tile_pool(name="work", bufs=1) as wk, \
             tc.tile_pool(name="small", bufs=2) as sm, \
             tc.tile_pool(name="ppA", bufs=3, space="PSUM") as ppA, \
             tc.tile_pool(name="ppB", bufs=2, space="PSUM") as ppB, \
             tc.tile_pool(name="ppR", bufs=2, space="PSUM") as ppR, \
             tc.tile_pool(name="ppC", bufs=1, space="PSUM") as ppC:

            # ============ SETUP ============
            id_f32 = sg.tile([128, 128], F32)
            make_identity(nc, id_f32)
            id_f32r = sg.tile([128, 128], F32R)
            nc.vector.tensor_copy(id_f32r, id_f32)

            eps_t = sg.tile([128, 1], F32)
            nc.vector.memset(eps_t, EPS)
            zrow = sg.tile([128, 1], F32)
            nc.vector.memset(zrow, 0.0)

            def zero_f32r(dst_ap, nfree):
                nc.vector.tensor_copy(dst_ap, _ap(zrow[:, 0:1], [[0, nfree]]))

            # weights [128 (d_in part), DT (d_in tile), D (d_out)] f32r:
            # DMA raw f32 bits into the f32r tile, then round in place.
            W = {}
            for mb in (0, 1):
                for qi, qn in enumerate(("q", "k", "v", "o")):
                    wr = sg.tile([128, DT, D], F32R, name=f"W{mb}{qn}")
                    src = p_d[f"{mb}Wqkv"][qi] if qn != "o" else p_d[f"{mb}Wo"]
                    stg = wk.tile([128, DT, D], F32, tag="xb", name=f"st{mb}{qn}")
                    nc.sync.dma_start(out=stg,
                                      in_=src.rearrange("(t p) d -> p t d", p=128))
                    if mb == 1 and qn == "k":
                        nc.scalar.mul(wr, stg, float(SCALE))
                    else:
                        nc.vector.tensor_copy(wr, stg)
                    W[f"{mb}{qn}"] = wr

            def pp_bias(src_ap, scale=None, name="b"):
                t = sg.tile([128, DT], F32, name=name)
                nc.sync.dma_start(out=t, in_=src_ap.rearrange("(t p) -> p t", p=128))
                if scale is not None:
                    nc.scalar.mul(t, t, float(scale))
                return t

            bk0_pp = pp_bias(p_d["0bqkv"][1], name="bk0")
            bq0s_pp = pp_bias(p_d["0bqkv"][0], SCALE, name="bq0s")
            bq1_pp = pp_bias(p_d["1bqkv"][0], name="bq1")
            bk1s_pp = pp_bias(p_d["1bqkv"][1], SCALE, name="bk1s")
            # ln1 gamma/beta applied feature-major (per-partition) at transpose time
            g1_pp = {mb: pp_bias(p_d[f"{mb}g1"], name=f"g1pp{mb}") for mb in (0, 1)}
            b1_pp = {mb: pp_bias(p_d[f"{mb}b1"], name=f"b1pp{mb}") for mb in (0, 1)}
            g1s_pp = pp_bias(p_d["1g1"], 1.0 / S, name="g1spp")   # for mean-pool fold
            b1_32 = pp_bias(p_d["1b1"], name="b1spp")

            def bcast(src_ap, name):
                row = sm.tile([1, D], F32, tag="bcrow", name=f"r_{name}")
                nc.sync.dma_start(out=row, in_=src_ap[None, :])
                t = sg.tile([128, D], F32, name=f"bc_{name}")
                nc.gpsimd.partition_broadcast(t, row)
                return t

            bo_bc = {mb: bcast(p_d[f"{mb}bo"], f"bo{mb}") for mb in (0, 1)}
            bv1_bc = bcast(p_d["1bqkv"][2], "bv1")
            ln_bc = {}
            for mb in (0, 1):
                for nm in ("g0", "b0"):
                    ln_bc[f"{mb}{nm}"] = bcast(p_d[f"{mb}{nm}"], f"ln{mb}{nm}")
            r0 = sm.tile([1, D], F32, tag="bcrow")
            nc.sync.dma_start(out=r0, in_=p_d["0bqkv"][0][None, :])
            r1 = sm.tile([1, D], F32, tag="bcrow")
            nc.sync.dma_start(out=r1, in_=p_d["0bqkv"][2][None, :])
            nc.vector.tensor_add(r0, r0, r1)
            bqv0_bc = sg.tile([128, D], F32)
            nc.gpsimd.partition_broadcast(bqv0_bc, r0)

            # I -> IT [128, DT, M] f32r (feature-major inducing points)
            Ib = sm.tile([M, D], F32, tag="bcrow", name="Ib")
            nc.sync.dma_start(out=Ib, in_=I_d[0])
            IT = sg.tile([128, DT, M], F32R)
            for m in range(DT):
                ps = ppB.tile([128, M], F32, tag="tp")
                nc.tensor.transpose(ps, Ib[:, 128 * m:128 * (m + 1)], id_f32[0:M, 0:M])
                nc.scalar.copy(IT[:, m, :], ps)

            # Q0T = (I @ Wq0 + bq0) * SCALE, feature-major
            Q0T = sg.tile([128, DT, M], F32R)
            for m in range(DT):
                ps = ppB.tile([128, M], F32, tag="tp")
                for k in range(DT):
                    nc.tensor.matmul(ps, W["0q"][:, k, 128 * m:128 * (m + 1)],
                                     IT[:, k, :], start=(k == 0), stop=(k == DT - 1))
                nc.scalar.activation(Q0T[:, m, :], ps, AF.Identity,
                                     bias=bq0s_pp[:, m:m + 1], scale=float(SCALE))

            # Q0blk [128, HP, 2M] block-diag (head pair) for scores0
            Q0blk = sg.tile([128, HP, 2 * M], BF16)
            zero_f32r(Q0blk, HP * 2 * M)
            for hp in range(HP):
                nc.vector.tensor_copy(Q0blk[0:64, hp, 0:M], Q0T[0:64, hp, :])
                nc.vector.tensor_copy(Q0blk[64:128, hp, M:2 * M], Q0T[64:128, hp, :])

            # Q0res_rep [128, D] f32: 8x-replicated (I @ Wq0 + bq0 + bv0), token-major
            ITrep = sg.tile([128, DT, 128], F32R)
            for k in range(DT):
                nc.vector.tensor_copy(ITrep[:, k, :],
                                      _ap(IT[:, k, :], [[0, 8], [1, M]]))
            psq = ppA.tile([128, D], F32, tag="lin")
            for k in range(DT):
                nc.tensor.matmul(psq, ITrep[:, k, :], W["0q"][:, k, :],
                                 start=(k == 0), stop=(k == DT - 1))
            Q0res_rep = sg.tile([128, D], F32)
            nc.vector.tensor_add(Q0res_rep, psq, bqv0_bc)

            # persistent zero-padded block-diag A^T stores (off-diag stays 0)
            AQ0 = sg.tile([128, 4, HP, 2, 4, M], BF16)  # [part(4g x 32k), j, hp, i, gcol, q]
            zero_f32r(AQ0, 4 * HP * 2 * 4 * M)
            AQ1 = sg.tile([128, H, 2, 4, S], BF16)      # [part(8b x 16k), h, half, bcol, q]
            zero_f32r(AQ1, H * 2 * 4 * S)
            A1sb = sg.tile([128, H, 2, M], BF16)        # [q-rows, h, batch-parity, k]
            zero_f32r(A1sb, H * 2 * M)
            # K1T variants with one head-parity's rows zeroed (so scores1 can
            # contract K=128 from row 0 and avoid the broken (64,96) PE quadrant)
            K1Te = sg.tile([128, DT, 256], BF16)
            zero_f32r(K1Te, DT * 256)
            K1To = sg.tile([128, DT, 256], BF16)
            zero_f32r(K1To, DT * 256)

            # ============ helpers ============
            def linear_fm(Wsb, inT, toks, bias_pp, tag, dt=F32R):
                outT = wk.tile([128, DT, 512], dt, tag=tag, bufs=2, name=f"fm_{tag}")
                for m in range(DT):
                    ps = ppA.tile([128, toks], F32, tag="lin")
                    for k in range(DT):
                        nc.tensor.matmul(ps, Wsb[:, k, 128 * m:128 * (m + 1)],
                                         inT[:, k, :toks], start=(k == 0), stop=(k == DT - 1))
                    if bias_pp is not None:
                        nc.scalar.activation(outT[:, m, :toks], ps, AF.Identity,
                                             bias=bias_pp[:, m:m + 1])
                    else:
                        nc.scalar.copy(outT[:, m, :toks], ps)
                return outT

            def layer_norm0(x_sb, g_bc, b_bc, otag):
                """Full token-major LN (gamma/beta broadcast tiles)."""
                st = sm.tile([128, 6], F32, tag="lnst")
                nc.vector.bn_stats(st, x_sb)
                mv = sm.tile([128, 2], F32, tag="lnmv")
                nc.vector.bn_aggr(mv, st)
                rstd = sm.tile([128, 1], F32, tag="lnr")
                nc.scalar.activation(rstd, mv[:, 1:2], AF.Sqrt, bias=eps_t[:, 0:1])
                nc.vector.reciprocal(rstd, rstd)
                t1 = sm.tile([128, D], F32, tag="lnt1")
                nc.vector.scalar_tensor_tensor(t1, x_sb, mv[:, 0:1], g_bc,
                                               op0=ALU.subtract, op1=ALU.mult)
                o = sm.tile([128, D], F32, tag=otag, name=f"ln_{otag}")
                nc.vector.scalar_tensor_tensor(o, t1, rstd[:, 0:1], b_bc,
                                               op0=ALU.mult, op1=ALU.add)
                return o

            def layer_norm1_pre(x_sb):
                """Normalize only ((x-mu)*rstd); gamma/beta folded into the
                feature-major transpose copy (per-partition there)."""
                st = sm.tile([128, 6], F32, tag="lnst")
                nc.vector.bn_stats(st, x_sb)
                mv = sm.tile([128, 2], F32, tag="lnmv")
                nc.vector.bn_aggr(mv, st)
                rstd = sm.tile([128, 1], F32, tag="lnr")
                nc.scalar.activation(rstd, mv[:, 1:2], AF.Sqrt, bias=eps_t[:, 0:1])
                nc.vector.reciprocal(rstd, rstd)
                o = sm.tile([128, D], F32, tag="lnN", name="lnN")
                nc.vector.tensor_scalar(o, x_sb, mv[:, 0:1], rstd[:, 0:1],
                                        op0=ALU.subtract, op1=ALU.mult)
                return o

            def transpose_fm_gb(isl, dst, dst_col0, gpp, bpp):
                """PE-transpose island [128 toks, 512] -> dst feature-major,
                applying per-partition gamma/beta in the PSUM->SBUF copy."""
                for m in range(DT):
                    ps = ppB.tile([128, 128], F32, tag="tp")
                    nc.tensor.transpose(ps, isl[:, 128 * m:128 * (m + 1)], id_f32)
                    nc.scalar.activation(dst[:, m, dst_col0:dst_col0 + 128], ps,
                                         AF.Identity, bias=bpp[:, m:m + 1],
                                         scale=gpp[:, m:m + 1])

            def transpose_fm(isl, dst, dst_col0):
                for m in range(DT):
                    ps = ppB.tile([128, 128], F32, tag="tp")
                    nc.tensor.transpose(ps, isl[:, 128 * m:128 * (m + 1)], id_f32)
                    nc.scalar.copy(dst[:, m, dst_col0:dst_col0 + 128], ps)

            # ============ one ISAB for one group of 16 batches ============
            def dummy_out(g):
                osb = sm.tile([G, D], F32, tag="osb", bufs=2)
                nc.vector.memset(osb, 0.0)
                nc.sync.dma_start(out=out_d[G * g:G * (g + 1), :], in_=osb)

            def isab(inT, g, last):
                # ---- mab0: Hm = MAB(I, X) ----
                if STAGE < 2:
                    return None
                KT = linear_fm(W["0k"], inT, 512, bk0_pp, tag="kt", dt=BF16)
                V0t = wk.tile([128, 4, D], BF16, tag="v0t", name="v0t")
                for i in range(4):
                    ps = ppA.tile([128, D], F32, tag="lin")
                    for k in range(DT):
                        nc.tensor.matmul(ps, inT[:, k, 128 * i:128 * (i + 1)],
                                         W["0v"][:, k, :], start=(k == 0), stop=(k == DT - 1))
                    nc.scalar.copy(V0t[:, i, :], ps)

                ps_s = ppA.tile([128, 512], F32, tag="lin")
                for hp in range(HP):
                    nc.tensor.matmul(ps_s[32 * hp:32 * (hp + 1), :], Q0blk[:, hp, :],
                                     KT[:, hp, :], start=True, stop=True,
                                     tile_position=(0, 32 * hp))
                E0 = sm.tile([128, 512], F32, tag="e0")
                nc.scalar.activation(E0, ps_s, AF.Exp)
                den = sm.tile([128, G], F32, tag="den0")
                nc.vector.tensor_reduce(den, E0.rearrange("p (b k) -> p b k", k=S),
                                        axis=AX.X, op=ALU.add)
                nc.vector.reciprocal(den, den)
                A0 = sm.tile([128, 512], BF16, tag="a0")
                nc.vector.tensor_tensor(
                    A0.rearrange("p (b k) -> p b k", k=S), E0.rearrange("p (b k) -> p b k", k=S),
                    _ap(den[:, :], [[1, G], [0, S]]), op=ALU.mult)
                # A^T for all 4 quads into the block-diag store
                A0v = A0.rearrange("p (j q k) -> p j q k", q=4, k=S)
                for hp in range(HP):
                    for gq in range(4):
                        nc.vector.transpose(
                            _ap(AQ0[32 * gq:32 * (gq + 1), 0, hp, 0, gq, :],
                                [[512, 4], [64, 2], [1, M]]),
                            A0v[32 * hp:32 * (hp + 1), :, gq, :])
                if STAGE < 3:
                    return None
                isl0 = []
                for jj in range(2):          # two islands of 128 tokens (8 batches)
                    ps_av = ppA.tile([128, D], F32, tag="lin")
                    for j2 in range(2):
                        j = 2 * jj + j2
                        for hp in range(HP):
                            for i in range(2):
                                h = 2 * hp + i
                                nc.tensor.matmul(
                                    ps_av[64 * j2:64 * j2 + 64, 64 * h:64 * (h + 1)],
                                    AQ0[:, j, hp, i, :, :].rearrange("p g q -> p (g q)"),
                                    V0t[:, j, 64 * h:64 * (h + 1)],
                                    start=True, stop=True, tile_position=(0, 64 * j2))
                    O0 = sm.tile([128, D], F32, tag="o0")
                    nc.vector.tensor_add(O0, ps_av, Q0res_rep)
                    L0 = layer_norm0(O0, ln_bc["0g0"], ln_bc["0b0"], otag="ln0out")
                    OT = wk.tile([128, DT, 128], F32R, tag="ot")
                    transpose_fm(L0, OT, 0)
                    psf = ppA.tile([128, D], F32, tag="lin")
                    for k in range(DT):
                        nc.tensor.matmul(psf, OT[:, k, :], W["0o"][:, k, :],
                                         start=(k == 0), stop=(k == DT - 1))
                    T2 = sm.tile([128, D], F32, tag="t2")
                    nc.vector.tensor_add(T2, psf, bo_bc[0])
                    O2 = sm.tile([128, D], F32, tag="o2")
                    nc.vector.scalar_tensor_tensor(O2, T2, 0.0, L0,
                                                   op0=ALU.max, op1=ALU.add)
                    isl0.append(layer_norm1_pre(O2))
                HT = wk.tile([128, DT, 272], F32R, tag="ht")
                for t2 in range(2):
                    transpose_fm_gb(isl0[t2], HT, 128 * t2, g1_pp[0], b1_pp[0])

                # ---- mab1: out = MAB(X, Hm) ----
                if STAGE < 4:
                    return None
                Q1T = wk.tile([128, DT, 512], F32R, tag="big", bufs=2, name="q1T")
                Q1Tb = wk.tile([128, DT, 512], BF16, tag="q1b", name="q1b")
                for m in range(DT):
                    ps = ppA.tile([128, 512], F32, tag="lin")
                    for k in range(DT):
                        nc.tensor.matmul(ps, W["1q"][:, k, 128 * m:128 * (m + 1)],
                                         inT[:, k, :], start=(k == 0), stop=(k == DT - 1))
                    nc.scalar.activation(Q1T[:, m, :], ps, AF.Identity,
                                         bias=bq1_pp[:, m:m + 1])
                    nc.vector.scalar_tensor_tensor(Q1Tb[:, m, :], ps, 1.0,
                                                   _ap(bq1_pp[:, m:m + 1], [[0, 512]]),
                                                   op0=ALU.mult, op1=ALU.add)
                for m in range(DT):
                    ps = ppA.tile([128, 256], F32, tag="lin")
                    for k in range(DT):
                        nc.tensor.matmul(ps, W["1k"][:, k, 128 * m:128 * (m + 1)],
                                         HT[:, k, :256], start=(k == 0), stop=(k == DT - 1))
                    nc.scalar.activation(K1Te[0:64, m, :], ps[0:64, :], AF.Identity,
                                         bias=bk1s_pp[0:64, m:m + 1])
                    nc.scalar.activation(K1To[64:128, m, :], ps[64:128, :], AF.Identity,
                                         bias=bk1s_pp[64:128, m:m + 1])
                V1t = wk.tile([128, 2, D], BF16, tag="v1t", name="v1t")
                for i in range(2):
                    ps = ppA.tile([128, D], F32, tag="lin")
                    for k in range(DT):
                        nc.tensor.matmul(ps, HT[:, k, 128 * i:128 * (i + 1)],
                                         W["1v"][:, k, :], start=(k == 0), stop=(k == DT - 1))
                    nc.scalar.copy(V1t[:, i, :], ps)

                if STAGE < 5:
                    return None
                if not last:
                    h1T = wk.tile([128, DT, 512], F32R, tag="fmX", bufs=2, name="h1T")
                else:
                    macc = sm.tile([128, DT, G], F32, tag="macc")
                for j in range(4):
                    j8, half = j // 2, j % 2
                    # scores for quad j: psum cols (h, batch-parity, k)
                    ps_s1 = ppB.tile([128, 256], F32, tag="tp")
                    for gq in range(4):
                        b = 4 * j + gq
                        for h in range(H):
                            hp, i = h // 2, h % 2
                            K1v = K1Te if i == 0 else K1To
                            nc.tensor.matmul(
                                ps_s1[32 * gq:32 * (gq + 1),
                                      32 * h + M * (gq % 2):32 * h + M * (gq % 2) + M],
                                Q1Tb[:, hp, 32 * b:32 * (b + 1)],
                                K1v[:, hp, M * b:M * (b + 1)],
                                start=True, stop=True, tile_position=(0, 32 * gq))
                    if STAGE == 41:
                        continue
                    E1 = sm.tile([128, 256], F32, tag="e1")
                    for gq in range(4):
                        sl = slice(32 * gq, 32 * (gq + 1))
                        off = M * (gq % 2)
                        nc.scalar.activation(
                            E1[sl].rearrange("p (h t) -> p h t", t=32)[:, :, off:off + M],
                            ps_s1[sl].rearrange("p (h t) -> p h t", t=32)[:, :, off:off + M],
                            AF.Exp)
                    den1 = sm.tile([128, H], F32, tag="den1")
                    for gq in range(4):
                        sl = slice(32 * gq, 32 * (gq + 1))
                        off = M * (gq % 2)
                        nc.vector.tensor_reduce(
                            den1[sl, :],
                            E1[sl].rearrange("p (h t) -> p h t", t=32)[:, :, off:off + M],
                            axis=AX.X, op=ALU.add)
                    nc.vector.reciprocal(den1, den1)
                    # normalize into A1sb (pad half per row-block stays zero)
                    for gq in range(4):
                        sl = slice(32 * gq, 32 * (gq + 1))
                        off = M * (gq % 2)
                        nc.vector.tensor_tensor(
                            A1sb[sl, :, gq % 2, :],
                            E1[sl].rearrange("p (h t) -> p h t", t=32)[:, :, off:off + M],
                            _ap(den1[sl, :], [[1, H], [0, M]]), op=ALU.mult)
                    if STAGE == 42:
                        continue
                    # A^T blocks: [32 q, (b2,k)] -> [32 (b2,k), q] at row 32*(2*half+gq//2)
                    for gq in range(4):
                        prow = 32 * (2 * half + gq // 2)
                        nc.vector.transpose(
                            _ap(AQ1[prow:prow + 32, 0, half, gq, :],
                                [[2 * 4 * S, H], [1, S]]),
                            A1sb[32 * gq:32 * (gq + 1), :, :, :])
                    if STAGE == 43:
                        continue
                    ps_av1 = ppA.tile([128, D], F32, tag="lin")
                    for h in range(H):
                        nc.tensor.matmul(ps_av1[:, 64 * h:64 * (h + 1)],
                                         AQ1[:, h, half, :, :].rearrange("p g q -> p (g q)"),
                                         V1t[:, j8, 64 * h:64 * (h + 1)],
                                         start=True, stop=True)
                    if STAGE == 44:
                        continue
                    ps_q1t = ppR.tile([128, D], F32R, tag="linr")
                    for m in range(DT):
                        nc.tensor.transpose(ps_q1t[:, 128 * m:128 * (m + 1)],
                                            Q1T[:, m, 128 * j:128 * (j + 1)], id_f32r)
                    if STAGE == 45:
                        continue
                    Tt = sm.tile([128, D], F32, tag="o0")
                    nc.vector.tensor_add(Tt, ps_av1, bv1_bc)
                    O1 = sm.tile([128, D], F32, tag="t2")
                    nc.vector.tensor_add(O1, Tt, ps_q1t)
                    L0 = layer_norm0(O1, ln_bc["1g0"], ln_bc["1b0"], otag="ln0out")
                    OT1 = wk.tile([128, DT, 128], F32R, tag="ot")
                    transpose_fm(L0, OT1, 0)
                    psf = ppA.tile([128, D], F32, tag="lin")
                    for k in range(DT):
                        nc.tensor.matmul(psf, OT1[:, k, :], W["1o"][:, k, :],
                                         start=(k == 0), stop=(k == DT - 1))
                    T2 = sm.tile([128, D], F32, tag="o2")
                    nc.vector.tensor_add(T2, psf, bo_bc[1])
                    O2 = sm.tile([128, D], F32, tag="o0")
                    nc.vector.scalar_tensor_tensor(O2, T2, 0.0, L0,
                                                   op0=ALU.max, op1=ALU.add)
                    OUTj = layer_norm1_pre(O2)
                    if not last:
                        transpose_fm_gb(OUTj, h1T, 128 * j, g1_pp[1], b1_pp[1])
                    else:
                        for m in range(DT):
                            ps = ppB.tile([128, 128], F32, tag="tp")
                            nc.tensor.transpose(ps, OUTj[:, 128 * m:128 * (m + 1)], id_f32)
                            nc.vector.tensor_reduce(macc[:, m, 4 * j:4 * (j + 1)],
                                                    ps.rearrange("p (b s) -> p b s", s=S),
                                                    axis=AX.X, op=ALU.add)
                if STAGE < 46:
                    return None
                if not last:
                    return h1T
                # pooled = g1/S * sum + b1, applied feature-major, then transpose out
                macc2 = sm.tile([128, DT, G], F32, tag="macc2")
                for m in range(DT):
                    nc.scalar.activation(macc2[:, m, :], macc[:, m, :], AF.Identity,
                                         bias=b1_32[:, m:m + 1], scale=g1s_pp[:, m:m + 1])
                osb = sm.tile([G, D], F32, tag="osb", bufs=2)
                for m in range(DT):
                    ps = ppC.tile([G, 128], F32, tag="mp")
                    nc.tensor.transpose(ps, macc2[:, m, :], id_f32)
                    nc.scalar.copy(osb[:, 128 * m:128 * (m + 1)], ps)
                nc.sync.dma_start(out=out_d[G * g:G * (g + 1), :], in_=osb)
                return None

            # ============ main loop ============
            x_flat = x_d.rearrange("b s d -> (b s) d")
            for g in range(ngroups):
                Xb = wk.tile([128, 4, D], F32, tag="xb")
                for i in range(4):
                    nc.sync.dma_start(
                        out=Xb[:, i, :],
                        in_=x_flat[512 * g + 128 * i: 512 * g + 128 * (i + 1), :])
                XT = wk.tile([128, DT, 512], F32R, tag="fmX", bufs=2, name="XT")
                for i in range(4):
                    for m in range(DT):
                        ps = ppB.tile([128, 128], F32, tag="tp")
                        nc.tensor.transpose(ps, Xb[:, i, 128 * m:128 * (m + 1)], id_f32)
                        nc.scalar.copy(XT[:, m, 128 * i:128 * (i + 1)], ps)
                h1T = isab(XT, g, last=False)
                if h1T is None or STAGE < 6:
                    dummy_out(g)
                    continue
                isab(h1T, g, last=True)

    nc.finalize()
    return nc


_CACHE = {}


def _get_nc(nb):
    if nb not in _CACHE:
        _CACHE[nb] = build(nb)
    return _CACHE[nb]


def kernel(**inputs):
    from concourse.bass_utils import run_bass_kernel_spmd

    x = np.ascontiguousarray(inputs["x"], dtype=np.float32)
    nbatch = x.shape[0]
    per = nbatch // NCORES
    nc = _get_nc(per)
    shared = {k: np.ascontiguousarray(np.asarray(v), dtype=np.float32)
              for k, v in inputs.items() if k != "x"}
    in_maps = [dict(shared, x=x[c * per:(c + 1) * per]) for c in range(NCORES)]
    res = run_bass_kernel_spmd(nc, in_maps, core_ids=list(range(NCORES)))
    return np.concatenate([r["out"] for r in res.results], axis=0)

